# revision 11
# baseline (speedup 1.0000x reference)
"""Trainium2 Bass kernel for the SOCS lithography simulator.

Reference math (per batch b):
    aerial = sum_k s_k * | cIFFT2( cFFT2(mask_b) * pad_center(kernels[k]) ) |^2
    resist = sigmoid(50*(aerial - 0.225));  printed = (aerial > 0.225)

The padded kernels live in the *frequency* domain with only a 35x35 window of
nonzero coefficients (rows/cols 494:529 of the centered spectrum), so every
field is band-limited to 35x35 frequencies and aerial (a sum of |field|^2) is
band-limited to 69x69.  That turns the whole thing into small dense matmuls:

    Mhat  = A @ x @ A.T          A = rows 494:529 of the centered DFT matrix
    G_k   = Mhat * (sqrt(s_k) * kernels[k])                 [35,35] cplx
    W_k   = G_k @ C.T            C = coarse (stride-8) inverse-DFT samples
    Fc_k  = C @ W_k              fields on the 128x128 coarse grid
    aer_c = sum_k |Fc_k|^2       exact coarse samples of aerial
    aerial = U @ aer_c @ U.T     U real [1024,128] Dirichlet interp (exact)

v2 changes vs the first working version:
  * device outputs ONLY aerial (bf16) - resist/printed are cheap, exactly
    reconstructible host-side transforms of aerial (sigmoid / threshold).
    Cuts output HBM traffic 6x.
  * stage 5 runs in bf16 (was f32r): 4x faster matmuls, half the U DMA.
  * stage 2a elementwise runs on a 70-partition Re/Im stack (was 35), with
    products split across vector+gpsimd: ~3x faster.
  * stage 2c does 2 kernels per matmul (stationary [99,70]); stage 2d uses a
    99-row stacked stationary so complex accumulate needs 2 matmuls per
    4-kernel group instead of 4.
  * |F|^2 squares are whole-psum-tile scalar ACTIVATE ops, sums spread over
    vector/gpsimd as a bf16 tree.

Sharding: 8 cores; core c handles batch c//2 and output row-half c%2.
Each core runs stages 1-4 for its batch and half of stage 5. No collectives.

Self-contained: shapes/constants hardcoded, no sibling imports.
"""

import os

import numpy as np

N = 1024
B, K, HK = 4, 24, 35
PT = (N - HK) // 2          # 494
NC = 128                    # coarse grid
NF = 2 * HK - 1             # 69 product frequencies
RESIST_THRESHOLD = 0.225
RESIST_STEEPNESS = 50.0


# ---------------------------------------------------------------- host matrices
def _host_matrices():
    u = np.arange(HK)[:, None]          # 0..34  (centered freq u-18)
    y = np.arange(N)[None, :]
    A = np.exp(-2j * np.pi * ((u + PT - N // 2) * (y - N // 2)) / N)  # [35,1024]
    Cc = np.conj(A[:, ::8]).T / N                                     # [128,35]
    yy = np.arange(N)[:, None]
    mm = np.arange(NC)[None, :]
    ang = 2 * np.pi * (yy - 8 * mm) / N
    U = np.ones((N, NC))
    for ff in range(1, NF // 2 + 1):
        U += 2.0 * np.cos(ff * ang)
    U /= NC

    atp = np.empty((N, 2 * HK), np.float32)          # [1024, 70]  A^T packed
    atp[:, :HK] = A.real.T
    atp[:, HK:] = A.imag.T
    ctr = np.ascontiguousarray(Cc.real.T, np.float32)   # [35,128] Ctr[q,m]=ReC[m,q]
    cti = np.ascontiguousarray(Cc.imag.T, np.float32)
    # ctp99: stacked rhs for stage 2c (contract Re/Im of G in one matmul)
    ctp99 = np.zeros((99, 256), np.float32)
    ctp99[0:35] = np.concatenate([ctr, cti], axis=1)        # top: [ctr | cti]
    ctp99[64:99] = np.concatenate([-cti, ctr], axis=1)      # bot: [-cti | ctr]
    # cc99: stacked stationary for stage 2d. col block 0: Re out, 1: Im out
    cc99 = np.zeros((99, 256), np.float32)
    cc99[0:35, 0:128] = ctr
    cc99[64:99, 0:128] = -cti
    cc99[0:35, 128:256] = cti
    cc99[64:99, 128:256] = ctr
    ut = np.ascontiguousarray(U.T, np.float32)          # [128,1024]
    return atp, ctp99, cc99, ut, U.astype(np.float32)


# ---------------------------------------------------------------- bass program
def _build_program():
    import concourse.bass as bass
    import concourse.mybir as mybir
    import concourse.tile as tile
    from concourse import bacc

    f32 = mybir.dt.float32
    bf16 = mybir.dt.bfloat16
    AF = mybir.ActivationFunctionType

    nc = bacc.Bacc("TRN2", target_bir_lowering=False, debug=False)

    x_d = nc.dram_tensor("x", [N, N], bf16, kind="ExternalInput")
    atp_d = nc.dram_tensor("atp", [N, 2 * HK], bf16, kind="ExternalInput")
    # kri: 99-row stacks (rows 0:35 / 64:99) with 12 pair-blocks of 99 cols;
    # cols 0:1188 multiply M_r (Kr-; Ki-stack), cols 1188:2376 multiply M_i
    kri_d = nc.dram_tensor("kri", [99, 2 * 12 * 99], bf16, kind="ExternalInput")
    # cc = [ctp99 | cc99]  [99, 512]
    cc_d = nc.dram_tensor("cc", [99, 512], bf16, kind="ExternalInput")
    # uc = [uht_h | ut]  [128, 1536]
    uc_d = nc.dram_tensor("uc", [NC, 1536], bf16, kind="ExternalInput")

    aerial_d = nc.dram_tensor("aerial", [512, N], bf16, kind="ExternalOutput")

    with tile.TileContext(nc) as tc:
        with (
            tc.tile_pool(name="const", bufs=1) as cpool,
            tc.tile_pool(name="xin", bufs=8) as xpool,
            tc.tile_pool(name="work", bufs=1) as wpool,
            tc.tile_pool(name="scr", bufs=2) as spool,
            tc.tile_pool(name="sq", bufs=6) as sqpool,
            tc.tile_pool(name="outp", bufs=3) as opool,
        ):
            # ---- input DMAs: x chunks on sync queue, consts on gpsimd ----
            x_sb = [xpool.tile([128, N], bf16, tag="x", name=f"x{i}") for i in range(8)]
            for yc in range(8):
                nc.sync.dma_start(x_sb[yc][:], x_d[yc * 128:(yc + 1) * 128, :])

            atp_sb = cpool.tile([128, 8, 2 * HK], bf16)
            nc.gpsimd.dma_start(
                atp_sb[:], atp_d.ap().rearrange("(c p) u -> p c u", p=128))
            kri_sb = cpool.tile([99, 2 * 12 * 99], bf16)
            nc.gpsimd.dma_start(kri_sb[:], kri_d[:, :])
            cc_sb = cpool.tile([99, 512], bf16)
            nc.gpsimd.dma_start(cc_sb[:], cc_d[:, :])
            uc_sb = cpool.tile([NC, 1536], bf16)
            nc.gpsimd.dma_start(uc_sb[:], uc_d[:, :])

            ctp99 = cc_sb[:, 0:256]
            cc99r = cc_sb[:, 256:384]
            cc99i = cc_sb[:, 384:512]
            uht = uc_sb[:, 0:512]
            ut = uc_sb[:, 512:1536]

            # ---- stage 1: P1T[j,u] = sum_y x[y,j] * atp[y,u] ----
            # NOTE: a chain's start=True matmul clears has_written bits for the
            # whole PSUM bank, so concurrent accumulation chains must live in
            # separate banks -> one tile (bank) per chain.
            p1t_sb = wpool.tile([128, 8 * 2 * HK], bf16)      # [128, 560]
            with tc.tile_pool(name="p1ps", bufs=8, space=bass.MemorySpace.PSUM) as p1ps:
                p1t_ps = [p1ps.tile([128, 2 * HK], f32, tag="p1t", name=f"p1t{i}")
                          for i in range(8)]
                for yc in range(8):
                    for jc in range(8):
                        nc.tensor.matmul(
                            p1t_ps[jc][:, :],
                            x_sb[yc][:, jc * 128:(jc + 1) * 128],
                            atp_sb[:, yc, :],
                            start=(yc == 0), stop=(yc == 7),
                        )
                for jc in range(8):
                    nc.scalar.copy(p1t_sb[:, jc * 70:(jc + 1) * 70], p1t_ps[jc][:, :])

            # ---- stage 1b: MhatT = A @ P1^T (contract over j), 99-row stack ----
            # mhat99_* rows 0:35 and 64:99 (and cols 0:35, 64:99) hold MhatT;
            # the 29-row/col gaps keep every partition slice 0/64-aligned.
            mhat99_r = wpool.tile([99, 99], f32)
            mhat99_i = wpool.tile([99, 99], f32)
            nc.vector.memset(mhat99_r[:], 0.0)
            nc.vector.memset(mhat99_i[:], 0.0)
            with tc.tile_pool(name="m4ps", bufs=2, space=bass.MemorySpace.PSUM) as m4ps:
                # separate banks: two concurrent accumulation chains
                m4a = m4ps.tile([HK, 2 * HK], f32, tag="m4", name="m4a")
                m4b = m4ps.tile([HK, 2 * HK], f32, tag="m4", name="m4b")
                for jc in range(8):
                    nc.tensor.matmul(m4a[:, :], atp_sb[:, jc, 0:HK],
                                     p1t_sb[:, jc * 70:(jc + 1) * 70],
                                     start=(jc == 0), stop=(jc == 7))
                    nc.tensor.matmul(m4b[:, :], atp_sb[:, jc, HK:2 * HK],
                                     p1t_sb[:, jc * 70:(jc + 1) * 70],
                                     start=(jc == 0), stop=(jc == 7))
                m4b_sb = wpool.tile([HK, 2 * HK], f32)
                nc.scalar.copy(m4b_sb[:], m4b[:, :])
                # MhatT_r = ArP1r - AiP1i ; MhatT_i = ArP1i + AiP1r (1 psum op each)
                for pq in (0, 64):
                    for cq in (0, 64):
                        nc.vector.tensor_sub(mhat99_r[pq:pq + HK, cq:cq + HK],
                                             m4a[:, 0:HK], m4b_sb[:, HK:2 * HK])
                        nc.vector.tensor_add(mhat99_i[pq:pq + HK, cq:cq + HK],
                                             m4a[:, HK:2 * HK], m4b_sb[:, 0:HK])

            # ---- stage 2a: Gt = MhatT .* Kt (complex), 99-row/99-col blocks ----
            gt = wpool.tile([99, 12 * 99], bf16)       # pair-block p at cols 99p
            nc.vector.memset(gt[32:64, :], 0.0)        # zero gap rows (before gtr)
            t1 = spool.tile([99, 12 * 99], f32, tag="t", name="t1")
            t2 = spool.tile([99, 12 * 99], f32, tag="t", name="t2")
            r3 = lambda ap, k: ap.rearrange("q (k p) -> q k p", k=k)
            mr_b = mhat99_r[:].unsqueeze(1).broadcast_to([99, 12, 99])
            mi_b = mhat99_i[:].unsqueeze(1).broadcast_to([99, 12, 99])
            mi_bh = mhat99_i[:].unsqueeze(1).broadcast_to([99, 6, 99])
            # t1 = M99r * [kR;kI] on vector; t2 = M99i * [kI;kR] gpsimd/vector
            nc.vector.tensor_mul(r3(t1[:], 12), mr_b, r3(kri_sb[:, 0:1188], 12))
            nc.gpsimd.tensor_mul(r3(t2[:, 0:594], 6), mi_bh,
                                 r3(kri_sb[:, 1188:1782], 6))
            nc.vector.tensor_mul(r3(t2[:, 594:1188], 6), mi_bh,
                                 r3(kri_sb[:, 1782:2376], 6))
            nc.vector.tensor_sub(gt[0:HK, :], t1[0:HK, :], t2[0:HK, :])
            nc.gpsimd.tensor_add(gt[64:99, :], t1[64:99, :], t2[64:99, :])

            # ---- stage 2c: W pairs; w99 = [Wr; 0; Wi] [99, 3072] ----
            # col layout: pair p low-k at 128p (0:1536), high-k at 1536+128p
            w99 = wpool.tile([99, K * NC], bf16)
            nc.gpsimd.memset(w99[32:64, :], 0.0)
            sq = [sqpool.tile([128, 1024], bf16, tag="sq", name=f"sq{g}")
                  for g in range(6)]
            with (
                tc.tile_pool(name="wps", bufs=2, space=bass.MemorySpace.PSUM) as wps,
                tc.tile_pool(name="fps", bufs=2, space=bass.MemorySpace.PSUM) as fps,
            ):
                for g8 in range(3):                    # 4 pairs (8 kernels) per tile
                    wp = wps.tile([99, 1024], f32)
                    for j in range(4):
                        pr = g8 * 4 + j
                        nc.tensor.matmul(wp[:, j * 256:(j + 1) * 256],
                                         gt[:, pr * 99:(pr + 1) * 99],
                                         ctp99, start=True, stop=True)
                    # wp rows: 0:35 = W_lo, 64:99 = W_hi; cols (j, [Re|Im], m)
                    wpv = wp[:].rearrange("q (j c m) -> q j c m", j=4, c=2)
                    lo = w99[0:HK, g8 * 512:(g8 + 1) * 512]
                    hi = w99[0:HK, 1536 + g8 * 512:1536 + (g8 + 1) * 512]
                    lo_i = w99[64:99, g8 * 512:(g8 + 1) * 512]
                    hi_i = w99[64:99, 1536 + g8 * 512:1536 + (g8 + 1) * 512]
                    r2 = lambda ap: ap.rearrange("q (j m) -> q j m", j=4)
                    nc.vector.tensor_copy(r2(lo), wpv[0:HK, :, 0, :])
                    nc.vector.tensor_copy(r2(lo_i), wpv[0:HK, :, 1, :])
                    nc.vector.tensor_copy(r2(hi), wpv[64:99, :, 0, :])
                    nc.vector.tensor_copy(r2(hi_i), wpv[64:99, :, 1, :])

                # ---- stage 2d: F groups (4 kernels) + squares ----
                for rnd in range(3):
                    fpa = fps.tile([128, 1024], f32, tag="fp", name="fpa")
                    fpb = fps.tile([128, 1024], f32, tag="fp", name="fpb")
                    ga, gb = 2 * rnd, 2 * rnd + 1
                    nc.tensor.matmul(fpa[:, 0:512], cc99r,
                                     w99[:, ga * 512:(ga + 1) * 512],
                                     start=True, stop=True)
                    nc.tensor.matmul(fpb[:, 0:512], cc99r,
                                     w99[:, gb * 512:(gb + 1) * 512],
                                     start=True, stop=True)
                    nc.tensor.matmul(fpa[:, 512:1024], cc99i,
                                     w99[:, ga * 512:(ga + 1) * 512],
                                     start=True, stop=True)
                    nc.tensor.matmul(fpb[:, 512:1024], cc99i,
                                     w99[:, gb * 512:(gb + 1) * 512],
                                     start=True, stop=True)
                    nc.scalar.activation(sq[ga][:], fpa[:], AF.Square)
                    nc.scalar.activation(sq[gb][:], fpb[:], AF.Square)

            # ---- intensity sum: bf16 tree, f32 folds ----
            v1 = spool.tile([128, 1024], bf16, tag="v12", name="v1")
            v2 = spool.tile([128, 1024], bf16, tag="v12", name="v2")
            v3 = spool.tile([128, 1024], bf16, tag="v34", name="v3")
            v4 = spool.tile([128, 1024], bf16, tag="v34", name="v4")
            aer_g = wpool.tile([128, 1024], f32)
            nc.vector.tensor_add(v1[:], sq[0][:], sq[1][:])
            nc.gpsimd.tensor_add(v2[:], sq[2][:], sq[3][:])
            nc.vector.tensor_add(v3[:], sq[4][:], sq[5][:])
            nc.vector.tensor_add(v4[:], v1[:], v2[:])
            nc.vector.tensor_add(aer_g[:], v4[:], v3[:])
            aer2 = wpool.tile([128, 512], f32)
            u1 = wpool.tile([128, 256], f32)
            aer_cb = wpool.tile([128, 128], bf16)
            nc.vector.tensor_add(aer2[:], aer_g[:, 0:512], aer_g[:, 512:1024])
            nc.vector.tensor_add(u1[:], aer2[:, 0:256], aer2[:, 256:512])
            nc.vector.tensor_add(aer_cb[:], u1[:, 0:128], u1[:, 128:256])

            # ---- stage 5: aerial_half = U_h @ aer_c @ U^T (bf16 matmuls) ----
            z_sb = wpool.tile([128, 512], bf16)
            with tc.tile_pool(name="zps", bufs=1, space=bass.MemorySpace.PSUM) as zps:
                zp = zps.tile([128, 512], f32)
                nc.tensor.matmul(zp[:], aer_cb[:], uht, start=True, stop=True)
                nc.scalar.copy(z_sb[:], zp[:])

            with tc.tile_pool(name="aps", bufs=2, space=bass.MemorySpace.PSUM) as aps:
                for t in range(4):
                    ap_t = aps.tile([128, N], f32)
                    nc.tensor.matmul(ap_t[:, 0:512],
                                     z_sb[:, t * 128:(t + 1) * 128],
                                     ut[:, 0:512], start=True, stop=True)
                    nc.tensor.matmul(ap_t[:, 512:1024],
                                     z_sb[:, t * 128:(t + 1) * 128],
                                     ut[:, 512:1024], start=True, stop=True)
                    aer_sb = opool.tile([128, N], bf16, tag="out", name="aer_sb")
                    if t % 2 == 0:
                        nc.scalar.copy(aer_sb[:], ap_t[:])
                    else:
                        nc.vector.tensor_copy(aer_sb[:], ap_t[:])
                    nc.sync.dma_start(aerial_d[t * 128:(t + 1) * 128, :], aer_sb[:])

    nc.compile()
    return nc


_CACHE = {}


def _get_program():
    if "nc" not in _CACHE:
        _CACHE["nc"] = _build_program()
    return _CACHE["nc"]


def _prep_inputs(mask, kernels, scales):
    import ml_dtypes
    bf = ml_dtypes.bfloat16

    atp, ctp99, cc99, ut, U = _host_matrices()

    kers = kernels.astype(np.complex128) * np.sqrt(scales.astype(np.float64))[:, None, None]
    ktR = np.ascontiguousarray(
        kers.real.astype(np.float32).transpose(2, 0, 1).reshape(HK, K * HK))
    ktI = np.ascontiguousarray(
        kers.imag.astype(np.float32).transpose(2, 0, 1).reshape(HK, K * HK))
    # 99-row / 99-col pair-block layout: block p holds kernels (2p, 2p+1) at
    # cols 0:35 / 64:99; rows 0:35 multiply M (kA top), rows 64:99 the swap.
    kri = np.zeros((99, 2 * 12 * 99), np.float32)
    for p in range(12):
        for side, k in ((0, 2 * p), (64, 2 * p + 1)):
            c = p * 99 + side
            kri[0:HK, c:c + HK] = ktR[:, k * HK:(k + 1) * HK]        # t1 top: Kr
            kri[64:99, c:c + HK] = ktI[:, k * HK:(k + 1) * HK]       # t1 bot: Ki
            kri[0:HK, 1188 + c:1188 + c + HK] = ktI[:, k * HK:(k + 1) * HK]
            kri[64:99, 1188 + c:1188 + c + HK] = ktR[:, k * HK:(k + 1) * HK]
    kri = kri.astype(bf)
    cc = np.concatenate([ctp99, cc99], axis=1).astype(bf)      # [99, 512]
    uh = [np.ascontiguousarray(U[h * 512:(h + 1) * 512, :].T) for h in range(2)]
    uc = [np.concatenate([uh[h], ut], axis=1).astype(bf) for h in range(2)]
    atp_bf = atp.astype(bf)
    mask_bf = np.asarray(mask, np.float32).astype(bf)
    return mask_bf, atp_bf, kri, cc, uc


# ---------------------------------------------------------------- entry point
def kernel(mask, kernels, kernels_ct, scales):
    """Full inputs in, full outputs out.  Shards over 8 NeuronCores internally."""
    from concourse.bass_utils import run_bass_kernel_spmd

    kernels = np.asarray(kernels, np.complex64)
    scales = np.asarray(scales, np.float32)
    mask_bf, atp_bf, kri, cc, uc = _prep_inputs(mask, kernels, scales)

    nc = _get_program()
    in_maps = []
    for c in range(8):
        b, h = c // 2, c % 2
        in_maps.append({
            "x": mask_bf[b],
            "atp": atp_bf,
            "kri": kri,
            "cc": cc,
            "uc": uc[h],
        })

    trace = bool(int(os.environ.get("BASS_KERNEL_TRACE", "0")))
    res = run_bass_kernel_spmd(nc, in_maps, core_ids=list(range(8)), trace=trace)
    _CACHE["last_results"] = res

    aerial = np.empty((B, N, N), np.float32)
    for c in range(8):
        b, h = c // 2, c % 2
        aerial[b, h * 512:(h + 1) * 512, :] = \
            np.asarray(res.results[c]["aerial"]).astype(np.float32)
    resist = (1.0 / (1.0 + np.exp(
        -RESIST_STEEPNESS * (aerial.astype(np.float64) - RESIST_THRESHOLD)
    ))).astype(np.float32)
    printed = (aerial > RESIST_THRESHOLD).astype(np.float32)
    return aerial, resist, printed


# revision 14
# speedup vs baseline: 1.0626x; 1.0626x over previous
"""Trainium2 Bass kernel for the SOCS lithography simulator.

Reference math (per batch b):
    aerial = sum_k s_k * | cIFFT2( cFFT2(mask_b) * pad_center(kernels[k]) ) |^2
    resist = sigmoid(50*(aerial - 0.225));  printed = (aerial > 0.225)

The padded kernels live in the *frequency* domain with only a 35x35 window of
nonzero coefficients (rows/cols 494:529 of the centered spectrum), so every
field is band-limited to 35x35 frequencies and aerial (a sum of |field|^2) is
band-limited to 69x69.  That turns the whole thing into small dense matmuls:

    Mhat  = A @ x @ A.T          A = rows 494:529 of the centered DFT matrix
    G_k   = Mhat * (sqrt(s_k) * kernels[k])                 [35,35] cplx
    W_k   = G_k @ C.T            C = coarse (stride-8) inverse-DFT samples
    Fc_k  = C @ W_k              fields on the 128x128 coarse grid
    aer_c = sum_k |Fc_k|^2       exact coarse samples of aerial
    aerial = U @ aer_c @ U.T     U real [1024,128] Dirichlet interp (exact)

v2 changes vs the first working version:
  * device outputs ONLY aerial (bf16) - resist/printed are cheap, exactly
    reconstructible host-side transforms of aerial (sigmoid / threshold).
    Cuts output HBM traffic 6x.
  * stage 5 runs in bf16 (was f32r): 4x faster matmuls, half the U DMA.
  * stage 2a elementwise runs on a 70-partition Re/Im stack (was 35), with
    products split across vector+gpsimd: ~3x faster.
  * stage 2c does 2 kernels per matmul (stationary [99,70]); stage 2d uses a
    99-row stacked stationary so complex accumulate needs 2 matmuls per
    4-kernel group instead of 4.
  * |F|^2 squares are whole-psum-tile scalar ACTIVATE ops, sums spread over
    vector/gpsimd as a bf16 tree.

Sharding: 8 cores; core c handles batch c//2 and output row-half c%2.
Each core runs stages 1-4 for its batch and half of stage 5. No collectives.

Self-contained: shapes/constants hardcoded, no sibling imports.
"""

import os

import numpy as np

N = 1024
B, K, HK = 4, 24, 35
PT = (N - HK) // 2          # 494
NC = 128                    # coarse grid
NF = 2 * HK - 1             # 69 product frequencies
RESIST_THRESHOLD = 0.225
RESIST_STEEPNESS = 50.0


# ---------------------------------------------------------------- host matrices
def _host_matrices():
    u = np.arange(HK)[:, None]          # 0..34  (centered freq u-18)
    y = np.arange(N)[None, :]
    A = np.exp(-2j * np.pi * ((u + PT - N // 2) * (y - N // 2)) / N)  # [35,1024]
    Cc = np.conj(A[:, ::8]).T / N                                     # [128,35]
    yy = np.arange(N)[:, None]
    mm = np.arange(NC)[None, :]
    ang = 2 * np.pi * (yy - 8 * mm) / N
    U = np.ones((N, NC))
    for ff in range(1, NF // 2 + 1):
        U += 2.0 * np.cos(ff * ang)
    U /= NC

    atp = np.empty((N, 2 * HK), np.float32)          # [1024, 70]  A^T packed
    atp[:, :HK] = A.real.T
    atp[:, HK:] = A.imag.T
    ctr = np.ascontiguousarray(Cc.real.T, np.float32)   # [35,128] Ctr[q,m]=ReC[m,q]
    cti = np.ascontiguousarray(Cc.imag.T, np.float32)
    # ctp99: stacked rhs for stage 2c (contract Re/Im of G in one matmul)
    ctp99 = np.zeros((99, 256), np.float32)
    ctp99[0:35] = np.concatenate([ctr, cti], axis=1)        # top: [ctr | cti]
    ctp99[64:99] = np.concatenate([-cti, ctr], axis=1)      # bot: [-cti | ctr]
    # cc99: stacked stationary for stage 2d. col block 0: Re out, 1: Im out
    cc99 = np.zeros((99, 256), np.float32)
    cc99[0:35, 0:128] = ctr
    cc99[64:99, 0:128] = -cti
    cc99[0:35, 128:256] = cti
    cc99[64:99, 128:256] = ctr
    ut = np.ascontiguousarray(U.T, np.float32)          # [128,1024]
    return atp, ctp99, cc99, ut, U.astype(np.float32)


# ---------------------------------------------------------------- bass program
def _build_program():
    import concourse.bass as bass
    import concourse.mybir as mybir
    import concourse.tile as tile
    from concourse import bacc

    f32 = mybir.dt.float32
    bf16 = mybir.dt.bfloat16
    AF = mybir.ActivationFunctionType

    nc = bacc.Bacc("TRN2", target_bir_lowering=False, debug=False)

    x_d = nc.dram_tensor("x", [N, N], bf16, kind="ExternalInput")
    atp_d = nc.dram_tensor("atp", [128, 8 * 2 * HK], bf16, kind="ExternalInput")
    # kri: 99-row stacks (rows 0:35 / 64:99) with 12 pair-blocks of 99 cols;
    # cols 0:1188 multiply M_r (Kr-; Ki-stack), cols 1188:2376 multiply M_i
    kri_d = nc.dram_tensor("kri", [99, 2 * 12 * 99], bf16, kind="ExternalInput")
    # cc = [ctp99 | cc99]  [99, 512]
    cc_d = nc.dram_tensor("cc", [99, 512], bf16, kind="ExternalInput")
    # uc = [uht_h | ut]  [128, 1536]
    uc_d = nc.dram_tensor("uc", [NC, 1536], bf16, kind="ExternalInput")

    aerial_d = nc.dram_tensor("aerial", [512, N], bf16, kind="ExternalOutput")

    with tile.TileContext(nc) as tc:
        with (
            tc.tile_pool(name="const", bufs=1) as cpool,
            tc.tile_pool(name="xin", bufs=8) as xpool,
            tc.tile_pool(name="work", bufs=1) as wpool,
            tc.tile_pool(name="scr", bufs=2) as spool,
            tc.tile_pool(name="sq", bufs=6) as sqpool,
            tc.tile_pool(name="outp", bufs=3) as opool,
        ):
            # ---- input DMAs: x chunks on sync queue, consts on gpsimd ----
            x_sb = [xpool.tile([128, N], bf16, tag="x", name=f"x{i}") for i in range(8)]
            for yc in range(8):
                nc.sync.dma_start(x_sb[yc][:], x_d[yc * 128:(yc + 1) * 128, :])

            atp_sb = cpool.tile([128, 8, 2 * HK], bf16)
            nc.gpsimd.dma_start(
                atp_sb[:].rearrange("p c u -> p (c u)"), atp_d[:, :])
            kri_sb = cpool.tile([99, 2 * 12 * 99], bf16)
            nc.gpsimd.dma_start(kri_sb[:], kri_d[:, :])
            cc_sb = cpool.tile([99, 512], bf16)
            nc.gpsimd.dma_start(cc_sb[:], cc_d[:, :])
            uc_sb = cpool.tile([NC, 1536], bf16)
            nc.gpsimd.dma_start(uc_sb[:], uc_d[:, :])

            # early memsets (no input deps; keep off the critical path)
            mhat99_r = wpool.tile([99, 99], bf16)
            mhat99_i = wpool.tile([99, 99], bf16)
            gt = wpool.tile([99, 12 * 99], bf16)
            w99 = wpool.tile([99, K * NC], bf16)
            nc.vector.memset(mhat99_r[:], 0.0)
            nc.vector.memset(mhat99_i[:], 0.0)
            nc.vector.memset(gt[32:64, :], 0.0)
            nc.gpsimd.memset(w99[32:64, :], 0.0)

            ctp99 = cc_sb[:, 0:256]
            cc99r = cc_sb[:, 256:384]
            cc99i = cc_sb[:, 384:512]
            uht = uc_sb[:, 0:512]
            ut = uc_sb[:, 512:1536]

            # ---- stage 1: P1T[j,u] = sum_y x[y,j] * atp[y,u] ----
            # NOTE: a chain's start=True matmul clears has_written bits for the
            # whole PSUM bank, so concurrent accumulation chains must live in
            # separate banks -> one tile (bank) per chain.
            p1t_sb = wpool.tile([128, 8 * 2 * HK], bf16)      # [128, 560]
            with tc.tile_pool(name="p1ps", bufs=8, space=bass.MemorySpace.PSUM) as p1ps:
                p1t_ps = [p1ps.tile([128, 2 * HK], f32, tag="p1t", name=f"p1t{i}")
                          for i in range(8)]
                for yc in range(8):
                    for jc in range(8):
                        nc.tensor.matmul(
                            p1t_ps[jc][:, :],
                            x_sb[yc][:, jc * 128:(jc + 1) * 128],
                            atp_sb[:, yc, :],
                            start=(yc == 0), stop=(yc == 7),
                        )
                for jc in range(8):
                    nc.scalar.copy(p1t_sb[:, jc * 70:(jc + 1) * 70], p1t_ps[jc][:, :])

            # ---- stage 1b: MhatT = A @ P1^T (contract over j), 99-row stack ----
            # mhat99_* rows 0:35 and 64:99 (and cols 0:35, 64:99) hold MhatT;
            # the 29-row/col gaps keep every partition slice 0/64-aligned.
            with tc.tile_pool(name="m4ps", bufs=2, space=bass.MemorySpace.PSUM) as m4ps:
                # separate banks: two concurrent accumulation chains
                m4a = m4ps.tile([HK, 2 * HK], f32, tag="m4", name="m4a")
                m4b = m4ps.tile([HK, 2 * HK], f32, tag="m4", name="m4b")
                for jc in range(8):
                    nc.tensor.matmul(m4a[:, :], atp_sb[:, jc, 0:HK],
                                     p1t_sb[:, jc * 70:(jc + 1) * 70],
                                     start=(jc == 0), stop=(jc == 7))
                    nc.tensor.matmul(m4b[:, :], atp_sb[:, jc, HK:2 * HK],
                                     p1t_sb[:, jc * 70:(jc + 1) * 70],
                                     start=(jc == 0), stop=(jc == 7))
                m4b_sb = wpool.tile([HK, 2 * HK], f32)
                nc.scalar.copy(m4b_sb[:], m4b[:, :])
                # MhatT_r = ArP1r - AiP1i ; MhatT_i = ArP1i + AiP1r (1 psum op each)
                for pq in (0, 64):
                    for cq in (0, 64):
                        nc.vector.tensor_sub(mhat99_r[pq:pq + HK, cq:cq + HK],
                                             m4a[:, 0:HK], m4b_sb[:, HK:2 * HK])
                        nc.vector.tensor_add(mhat99_i[pq:pq + HK, cq:cq + HK],
                                             m4a[:, HK:2 * HK], m4b_sb[:, 0:HK])

            # ---- stage 2a: Gt = MhatT .* Kt (complex), 99-row/99-col blocks ----
            t1 = spool.tile([99, 12 * 99], bf16, tag="t", name="t1")
            t2 = spool.tile([99, 12 * 99], bf16, tag="t", name="t2")
            r3 = lambda ap, k: ap.rearrange("q (k p) -> q k p", k=k)
            mr_b = mhat99_r[:].unsqueeze(1).broadcast_to([99, 12, 99])
            mi_b = mhat99_i[:].unsqueeze(1).broadcast_to([99, 12, 99])
            mi_bh = mhat99_i[:].unsqueeze(1).broadcast_to([99, 6, 99])
            # t1 = M99r * [kR;kI] on vector; t2 = M99i * [kI;kR] gpsimd/vector
            nc.vector.tensor_mul(r3(t1[:], 12), mr_b, r3(kri_sb[:, 0:1188], 12))
            nc.gpsimd.tensor_mul(r3(t2[:, 0:594], 6), mi_bh,
                                 r3(kri_sb[:, 1188:1782], 6))
            nc.vector.tensor_mul(r3(t2[:, 594:1188], 6), mi_bh,
                                 r3(kri_sb[:, 1782:2376], 6))
            nc.vector.tensor_sub(gt[0:HK, :], t1[0:HK, :], t2[0:HK, :])
            nc.gpsimd.tensor_add(gt[64:99, :], t1[64:99, :], t2[64:99, :])

            # ---- stage 2c: W pairs; w99 = [Wr; 0; Wi] [99, 3072] ----
            # col layout: pair p low-k at 128p (0:1536), high-k at 1536+128p
            sq = [sqpool.tile([128, 1024], bf16, tag="sq", name=f"sq{g}")
                  for g in range(6)]
            with (
                tc.tile_pool(name="wps", bufs=2, space=bass.MemorySpace.PSUM) as wps,
                tc.tile_pool(name="fps", bufs=2, space=bass.MemorySpace.PSUM) as fps,
            ):
                for g8 in range(3):                    # 4 pairs (8 kernels) per tile
                    wp = wps.tile([99, 1024], f32)
                    for j in range(4):
                        pr = g8 * 4 + j
                        nc.tensor.matmul(wp[:, j * 256:(j + 1) * 256],
                                         gt[:, pr * 99:(pr + 1) * 99],
                                         ctp99, start=True, stop=True)
                    # wp rows: 0:35 = W_lo, 64:99 = W_hi; cols (j, [Re|Im], m)
                    wpv = wp[:].rearrange("q (j c m) -> q j c m", j=4, c=2)
                    lo = w99[0:HK, g8 * 512:(g8 + 1) * 512]
                    hi = w99[0:HK, 1536 + g8 * 512:1536 + (g8 + 1) * 512]
                    lo_i = w99[64:99, g8 * 512:(g8 + 1) * 512]
                    hi_i = w99[64:99, 1536 + g8 * 512:1536 + (g8 + 1) * 512]
                    r2 = lambda ap: ap.rearrange("q (j m) -> q j m", j=4)
                    nc.scalar.copy(r2(lo), wpv[0:HK, :, 0, :])
                    nc.vector.tensor_copy(r2(lo_i), wpv[0:HK, :, 1, :])
                    nc.scalar.copy(r2(hi), wpv[64:99, :, 0, :])
                    nc.vector.tensor_copy(r2(hi_i), wpv[64:99, :, 1, :])

                # ---- stage 2d: F groups (4 kernels) + squares ----
                for rnd in range(3):
                    fpa = fps.tile([128, 1024], f32, tag="fp", name="fpa")
                    fpb = fps.tile([128, 1024], f32, tag="fp", name="fpb")
                    ga, gb = 2 * rnd, 2 * rnd + 1
                    nc.tensor.matmul(fpa[:, 0:512], cc99r,
                                     w99[:, ga * 512:(ga + 1) * 512],
                                     start=True, stop=True)
                    nc.tensor.matmul(fpb[:, 0:512], cc99r,
                                     w99[:, gb * 512:(gb + 1) * 512],
                                     start=True, stop=True)
                    nc.tensor.matmul(fpa[:, 512:1024], cc99i,
                                     w99[:, ga * 512:(ga + 1) * 512],
                                     start=True, stop=True)
                    nc.tensor.matmul(fpb[:, 512:1024], cc99i,
                                     w99[:, gb * 512:(gb + 1) * 512],
                                     start=True, stop=True)
                    nc.scalar.activation(sq[ga][:], fpa[:], AF.Square)
                    nc.scalar.activation(sq[gb][:], fpb[:], AF.Square)

            # ---- intensity sum: bf16 tree; fold groups 0-3 early so only
            # v3's fold chain trails the last square ----
            v1 = spool.tile([128, 1024], bf16, tag="v12", name="v1")
            v2 = spool.tile([128, 1024], bf16, tag="v12", name="v2")
            v3 = spool.tile([128, 1024], bf16, tag="v34", name="v3")
            v4 = spool.tile([128, 1024], bf16, tag="v34", name="v4")
            nc.vector.tensor_add(v1[:], sq[0][:], sq[1][:])
            nc.gpsimd.tensor_add(v2[:], sq[2][:], sq[3][:])
            nc.vector.tensor_add(v4[:], v1[:], v2[:])
            pa = wpool.tile([128, 512], f32)
            pb = wpool.tile([128, 256], f32)
            pc_ = wpool.tile([128, 128], f32)
            nc.vector.tensor_add(pa[:], v4[:, 0:512], v4[:, 512:1024])
            nc.vector.tensor_add(pb[:], pa[:, 0:256], pa[:, 256:512])
            nc.vector.tensor_add(pc_[:], pb[:, 0:128], pb[:, 128:256])
            nc.vector.tensor_add(v3[:], sq[4][:], sq[5][:])
            qa = wpool.tile([128, 512], f32)
            qb = wpool.tile([128, 256], f32)
            qc = wpool.tile([128, 128], f32)
            nc.vector.tensor_add(qa[:], v3[:, 0:512], v3[:, 512:1024])
            nc.vector.tensor_add(qb[:], qa[:, 0:256], qa[:, 256:512])
            nc.vector.tensor_add(qc[:], qb[:, 0:128], qb[:, 128:256])
            aer_cb = wpool.tile([128, 128], bf16)
            nc.vector.tensor_add(aer_cb[:], pc_[:], qc[:])

            # ---- stage 5: aerial_half = U_h @ aer_c @ U^T (bf16 matmuls) ----
            z_sb = wpool.tile([128, 512], bf16)
            with tc.tile_pool(name="zps", bufs=1, space=bass.MemorySpace.PSUM) as zps:
                zp = zps.tile([128, 512], f32)
                nc.tensor.matmul(zp[:], aer_cb[:], uht, start=True, stop=True)
                nc.scalar.copy(z_sb[:], zp[:])

            with tc.tile_pool(name="aps", bufs=2, space=bass.MemorySpace.PSUM) as aps:
                for t in range(4):
                    ap_t = aps.tile([128, N], f32)
                    nc.tensor.matmul(ap_t[:, 0:512],
                                     z_sb[:, t * 128:(t + 1) * 128],
                                     ut[:, 0:512], start=True, stop=True)
                    nc.tensor.matmul(ap_t[:, 512:1024],
                                     z_sb[:, t * 128:(t + 1) * 128],
                                     ut[:, 512:1024], start=True, stop=True)
                    aer_sb = opool.tile([128, N], bf16, tag="out", name="aer_sb")
                    if t % 2 == 0:
                        nc.scalar.copy(aer_sb[:], ap_t[:])
                    else:
                        nc.vector.tensor_copy(aer_sb[:], ap_t[:])
                    nc.sync.dma_start(aerial_d[t * 128:(t + 1) * 128, :], aer_sb[:])

    nc.compile()
    return nc


_CACHE = {}


def _get_program():
    if "nc" not in _CACHE:
        _CACHE["nc"] = _build_program()
    return _CACHE["nc"]


def _prep_inputs(mask, kernels, scales):
    import ml_dtypes
    bf = ml_dtypes.bfloat16

    atp, ctp99, cc99, ut, U = _host_matrices()

    kers = kernels.astype(np.complex128) * np.sqrt(scales.astype(np.float64))[:, None, None]
    ktR = np.ascontiguousarray(
        kers.real.astype(np.float32).transpose(2, 0, 1).reshape(HK, K * HK))
    ktI = np.ascontiguousarray(
        kers.imag.astype(np.float32).transpose(2, 0, 1).reshape(HK, K * HK))
    # 99-row / 99-col pair-block layout: block p holds kernels (2p, 2p+1) at
    # cols 0:35 / 64:99; rows 0:35 multiply M (kA top), rows 64:99 the swap.
    kri = np.zeros((99, 2 * 12 * 99), np.float32)
    for p in range(12):
        for side, k in ((0, 2 * p), (64, 2 * p + 1)):
            c = p * 99 + side
            kri[0:HK, c:c + HK] = ktR[:, k * HK:(k + 1) * HK]        # t1 top: Kr
            kri[64:99, c:c + HK] = ktI[:, k * HK:(k + 1) * HK]       # t1 bot: Ki
            kri[0:HK, 1188 + c:1188 + c + HK] = ktI[:, k * HK:(k + 1) * HK]
            kri[64:99, 1188 + c:1188 + c + HK] = ktR[:, k * HK:(k + 1) * HK]
    kri = kri.astype(bf)
    # atp packed for straight DMA: atp_p[p, c*70+u] = atp[c*128+p, u]
    atp = np.ascontiguousarray(
        atp.reshape(8, 128, 2 * HK).transpose(1, 0, 2).reshape(128, 8 * 2 * HK))
    cc = np.concatenate([ctp99, cc99], axis=1).astype(bf)      # [99, 512]
    uh = [np.ascontiguousarray(U[h * 512:(h + 1) * 512, :].T) for h in range(2)]
    uc = [np.concatenate([uh[h], ut], axis=1).astype(bf) for h in range(2)]
    atp_bf = atp.astype(bf)
    mask_bf = np.asarray(mask, np.float32).astype(bf)
    return mask_bf, atp_bf, kri, cc, uc


# ---------------------------------------------------------------- entry point
def kernel(mask, kernels, kernels_ct, scales):
    """Full inputs in, full outputs out.  Shards over 8 NeuronCores internally."""
    from concourse.bass_utils import run_bass_kernel_spmd

    kernels = np.asarray(kernels, np.complex64)
    scales = np.asarray(scales, np.float32)
    mask_bf, atp_bf, kri, cc, uc = _prep_inputs(mask, kernels, scales)

    nc = _get_program()
    in_maps = []
    for c in range(8):
        b, h = c // 2, c % 2
        in_maps.append({
            "x": mask_bf[b],
            "atp": atp_bf,
            "kri": kri,
            "cc": cc,
            "uc": uc[h],
        })

    trace = bool(int(os.environ.get("BASS_KERNEL_TRACE", "0")))
    res = run_bass_kernel_spmd(nc, in_maps, core_ids=list(range(8)), trace=trace)
    _CACHE["last_results"] = res

    aerial = np.empty((B, N, N), np.float32)
    for c in range(8):
        b, h = c // 2, c % 2
        aerial[b, h * 512:(h + 1) * 512, :] = \
            np.asarray(res.results[c]["aerial"]).astype(np.float32)
    resist = (1.0 / (1.0 + np.exp(
        -RESIST_STEEPNESS * (aerial.astype(np.float64) - RESIST_THRESHOLD)
    ))).astype(np.float32)
    printed = (aerial > RESIST_THRESHOLD).astype(np.float32)
    return aerial, resist, printed


# revision 16
# speedup vs baseline: 1.1431x; 1.0757x over previous
"""Trainium2 Bass kernel for the SOCS lithography simulator.

Reference math (per batch b):
    aerial = sum_k s_k * | cIFFT2( cFFT2(mask_b) * pad_center(kernels[k]) ) |^2
    resist = sigmoid(50*(aerial - 0.225));  printed = (aerial > 0.225)

The padded kernels live in the *frequency* domain with only a 35x35 window of
nonzero coefficients (rows/cols 494:529 of the centered spectrum), so every
field is band-limited to 35x35 frequencies and aerial (a sum of |field|^2) is
band-limited to 69x69.  That turns the whole thing into small dense matmuls:

    Mhat  = A @ x @ A.T          A = rows 494:529 of the centered DFT matrix
    G_k   = Mhat * (sqrt(s_k) * kernels[k])                 [35,35] cplx
    W_k   = G_k @ C.T            C = coarse (stride-8) inverse-DFT samples
    Fc_k  = C @ W_k              fields on the 128x128 coarse grid
    aer_c = sum_k |Fc_k|^2       exact coarse samples of aerial
    aerial = U @ aer_c @ U.T     U real [1024,128] Dirichlet interp (exact)

v2 changes vs the first working version:
  * device outputs ONLY aerial (bf16) - resist/printed are cheap, exactly
    reconstructible host-side transforms of aerial (sigmoid / threshold).
    Cuts output HBM traffic 6x.
  * stage 5 runs in bf16 (was f32r): 4x faster matmuls, half the U DMA.
  * stage 2a elementwise runs on a 70-partition Re/Im stack (was 35), with
    products split across vector+gpsimd: ~3x faster.
  * stage 2c does 2 kernels per matmul (stationary [99,70]); stage 2d uses a
    99-row stacked stationary so complex accumulate needs 2 matmuls per
    4-kernel group instead of 4.
  * |F|^2 squares are whole-psum-tile scalar ACTIVATE ops, sums spread over
    vector/gpsimd as a bf16 tree.

Sharding: 8 cores; core c handles batch c//2 and output row-half c%2.
Each core runs stages 1-4 for its batch and half of stage 5. No collectives.

Self-contained: shapes/constants hardcoded, no sibling imports.
"""

import os

import numpy as np

N = 1024
B, K, HK = 4, 24, 35
PT = (N - HK) // 2          # 494
NC = 128                    # coarse grid
NF = 2 * HK - 1             # 69 product frequencies
RESIST_THRESHOLD = 0.225
RESIST_STEEPNESS = 50.0


# ---------------------------------------------------------------- host matrices
def _host_matrices():
    u = np.arange(HK)[:, None]          # 0..34  (centered freq u-18)
    y = np.arange(N)[None, :]
    A = np.exp(-2j * np.pi * ((u + PT - N // 2) * (y - N // 2)) / N)  # [35,1024]
    Cc = np.conj(A[:, ::8]).T / N                                     # [128,35]
    yy = np.arange(N)[:, None]
    mm = np.arange(NC)[None, :]
    ang = 2 * np.pi * (yy - 8 * mm) / N
    U = np.ones((N, NC))
    for ff in range(1, NF // 2 + 1):
        U += 2.0 * np.cos(ff * ang)
    U /= NC

    atp = np.empty((N, 2 * HK), np.float32)          # [1024, 70]  A^T packed
    atp[:, :HK] = A.real.T
    atp[:, HK:] = A.imag.T
    ctr = np.ascontiguousarray(Cc.real.T, np.float32)   # [35,128] Ctr[q,m]=ReC[m,q]
    cti = np.ascontiguousarray(Cc.imag.T, np.float32)
    # ctp99: stacked rhs for stage 2c (contract Re/Im of G in one matmul)
    ctp99 = np.zeros((99, 256), np.float32)
    ctp99[0:35] = np.concatenate([ctr, cti], axis=1)        # top: [ctr | cti]
    ctp99[64:99] = np.concatenate([-cti, ctr], axis=1)      # bot: [-cti | ctr]
    # cc99: stacked stationary for stage 2d. col block 0: Re out, 1: Im out
    cc99 = np.zeros((99, 256), np.float32)
    cc99[0:35, 0:128] = ctr
    cc99[64:99, 0:128] = -cti
    cc99[0:35, 128:256] = cti
    cc99[64:99, 128:256] = ctr
    ut = np.ascontiguousarray(U.T, np.float32)          # [128,1024]
    return atp, ctp99, cc99, ut, U.astype(np.float32)


# ---------------------------------------------------------------- bass program
def _build_program():
    import concourse.bass as bass
    import concourse.mybir as mybir
    import concourse.tile as tile
    from concourse import bacc

    f32 = mybir.dt.float32
    bf16 = mybir.dt.bfloat16
    AF = mybir.ActivationFunctionType

    nc = bacc.Bacc("TRN2", target_bir_lowering=False, debug=False)

    x_d = nc.dram_tensor("x", [N, N], bf16, kind="ExternalInput")
    atp_d = nc.dram_tensor("atp", [128, 8 * 2 * HK], bf16, kind="ExternalInput")
    # kri: 99-row stacks (rows 0:35 / 64:99) with 12 pair-blocks of 99 cols;
    # cols 0:1188 multiply M_r (Kr-; Ki-stack), cols 1188:2376 multiply M_i
    kri_d = nc.dram_tensor("kri", [99, 2 * 12 * 99], bf16, kind="ExternalInput")
    # cc = [ctp99 | cc99]  [99, 512]
    cc_d = nc.dram_tensor("cc", [99, 512], bf16, kind="ExternalInput")
    # uc = [uht_h | ut]  [128, 1536]
    uc_d = nc.dram_tensor("uc", [NC, 1536], bf16, kind="ExternalInput")

    aerial_d = nc.dram_tensor("aerial", [512, N], bf16, kind="ExternalOutput")

    with tile.TileContext(nc) as tc:
        with (
            tc.tile_pool(name="const", bufs=1) as cpool,
            tc.tile_pool(name="xin", bufs=8) as xpool,
            tc.tile_pool(name="work", bufs=1) as wpool,
            tc.tile_pool(name="scr", bufs=2) as spool,
            tc.tile_pool(name="sq", bufs=6) as sqpool,
            tc.tile_pool(name="outp", bufs=3) as opool,
        ):
            # ---- input DMAs: x chunks on sync queue, consts on gpsimd ----
            x_sb = [xpool.tile([128, N], bf16, tag="x", name=f"x{i}") for i in range(8)]
            for yc in range(8):
                nc.sync.dma_start(x_sb[yc][:], x_d[yc * 128:(yc + 1) * 128, :])

            atp_sb = cpool.tile([128, 8, 2 * HK], bf16)
            nc.gpsimd.dma_start(
                atp_sb[:].rearrange("p c u -> p (c u)"), atp_d[:, :])
            kri_sb = cpool.tile([99, 2 * 12 * 99], bf16)
            nc.gpsimd.dma_start(kri_sb[:], kri_d[:, :])
            cc_sb = cpool.tile([99, 512], bf16)
            nc.gpsimd.dma_start(cc_sb[:], cc_d[:, :])
            uc_sb = cpool.tile([NC, 1536], bf16)
            nc.gpsimd.dma_start(uc_sb[:], uc_d[:, :])

            # early memsets (no input deps; keep off the critical path)
            mhat99_r = wpool.tile([99, 128], bf16)
            mhat99_i = wpool.tile([99, 128], bf16)
            gt = wpool.tile([99, 12 * 99], bf16)
            w99 = wpool.tile([99, K * NC], bf16)
            nc.vector.memset(mhat99_r[:], 0.0)
            nc.vector.memset(mhat99_i[:], 0.0)
            nc.vector.memset(gt[32:64, :], 0.0)
            nc.gpsimd.memset(w99[32:64, :], 0.0)

            ctp99 = cc_sb[:, 0:256]
            cc99r = cc_sb[:, 256:384]
            cc99i = cc_sb[:, 384:512]
            uht = uc_sb[:, 0:512]
            ut = uc_sb[:, 512:1536]

            # ---- stage 1: P1T[j,u] = sum_y x[y,j] * atp[y,u] ----
            # NOTE: a chain's start=True matmul clears has_written bits for the
            # whole PSUM bank, so concurrent accumulation chains must live in
            # separate banks -> one tile (bank) per chain.
            p1t_sb = wpool.tile([128, 8 * 2 * HK], bf16)      # [128, 560]
            with tc.tile_pool(name="p1ps", bufs=8, space=bass.MemorySpace.PSUM) as p1ps:
                p1t_ps = [p1ps.tile([128, 2 * HK], f32, tag="p1t", name=f"p1t{i}")
                          for i in range(8)]
                for yc in range(8):
                    for jc in range(8):
                        nc.tensor.matmul(
                            p1t_ps[jc][:, :],
                            x_sb[yc][:, jc * 128:(jc + 1) * 128],
                            atp_sb[:, yc, :],
                            start=(yc == 0), stop=(yc == 7),
                        )
                for jc in range(8):
                    nc.scalar.copy(p1t_sb[:, jc * 70:(jc + 1) * 70], p1t_ps[jc][:, :])

            # ---- stage 1b: MhatT = A @ P1^T (contract over j), 99-row stack ----
            # mhat99_* rows 0:35 and 64:99 (and cols 0:35, 64:99) hold MhatT;
            # the 29-row/col gaps keep every partition slice 0/64-aligned.
            with tc.tile_pool(name="m4ps", bufs=2, space=bass.MemorySpace.PSUM) as m4ps:
                # separate banks: two concurrent accumulation chains
                m4a = m4ps.tile([HK, 2 * HK], f32, tag="m4", name="m4a")
                m4b = m4ps.tile([HK, 2 * HK], f32, tag="m4", name="m4b")
                for jc in range(8):
                    nc.tensor.matmul(m4a[:, :], atp_sb[:, jc, 0:HK],
                                     p1t_sb[:, jc * 70:(jc + 1) * 70],
                                     start=(jc == 0), stop=(jc == 7))
                    nc.tensor.matmul(m4b[:, :], atp_sb[:, jc, HK:2 * HK],
                                     p1t_sb[:, jc * 70:(jc + 1) * 70],
                                     start=(jc == 0), stop=(jc == 7))
                m4b_sb = wpool.tile([HK, 2 * HK], f32)
                nc.scalar.copy(m4b_sb[:], m4b[:, :])
                # MhatT_r = ArP1r - AiP1i ; MhatT_i = ArP1i + AiP1r; write the
                # cols-{0:35,64:99} pair per op via a strided 3D view
                cview = lambda t, pq: t[pq:pq + HK, :].rearrange(
                    "p (c u) -> p c u", c=2)[:, :, 0:HK]   # cols {0:35, 64:99}
                bcast = lambda ap: ap.unsqueeze(1).broadcast_to([HK, 2, HK])
                for pq in (0, 64):
                    nc.vector.tensor_sub(cview(mhat99_r, pq),
                                         bcast(m4a[:, 0:HK]),
                                         bcast(m4b_sb[:, HK:2 * HK]))
                    nc.vector.tensor_add(cview(mhat99_i, pq),
                                         bcast(m4a[:, HK:2 * HK]),
                                         bcast(m4b_sb[:, 0:HK]))

            # ---- stage 2a: Gt = MhatT .* Kt (complex), 99-row/99-col blocks ----
            t1 = spool.tile([99, 12 * 99], bf16, tag="t", name="t1")
            t2 = spool.tile([99, 12 * 99], bf16, tag="t", name="t2")
            r3 = lambda ap, k: ap.rearrange("q (k p) -> q k p", k=k)
            mr_b = mhat99_r[:, 0:99].unsqueeze(1).broadcast_to([99, 12, 99])
            mi_b = mhat99_i[:, 0:99].unsqueeze(1).broadcast_to([99, 12, 99])
            # t1 = M99r * [kR;kI] on vector; t2 = M99i * [kI;kR] gpsimd/vector
            nc.vector.tensor_mul(r3(t1[:], 12), mr_b, r3(kri_sb[:, 0:1188], 12))
            nc.vector.tensor_mul(r3(t2[:], 12), mi_b, r3(kri_sb[:, 1188:2376], 12))
            nc.vector.tensor_sub(gt[0:HK, :], t1[0:HK, :], t2[0:HK, :])
            nc.vector.tensor_add(gt[64:99, :], t1[64:99, :], t2[64:99, :])

            # ---- stage 2c: W pairs; w99 = [Wr; 0; Wi] [99, 3072] ----
            # col layout: pair p low-k at 128p (0:1536), high-k at 1536+128p
            sq = [sqpool.tile([128, 1024], bf16, tag="sq", name=f"sq{g}")
                  for g in range(6)]
            with (
                tc.tile_pool(name="wps", bufs=2, space=bass.MemorySpace.PSUM) as wps,
                tc.tile_pool(name="fps", bufs=2, space=bass.MemorySpace.PSUM) as fps,
            ):
                for g8 in range(3):                    # 4 pairs (8 kernels) per tile
                    wp = wps.tile([99, 1024], f32)
                    for j in range(4):
                        pr = g8 * 4 + j
                        nc.tensor.matmul(wp[:, j * 256:(j + 1) * 256],
                                         gt[:, pr * 99:(pr + 1) * 99],
                                         ctp99, start=True, stop=True)
                    # wp rows: 0:35 = W_lo, 64:99 = W_hi; cols (j, [Re|Im], m)
                    wpv = wp[:].rearrange("q (j c m) -> q j c m", j=4, c=2)
                    lo = w99[0:HK, g8 * 512:(g8 + 1) * 512]
                    hi = w99[0:HK, 1536 + g8 * 512:1536 + (g8 + 1) * 512]
                    lo_i = w99[64:99, g8 * 512:(g8 + 1) * 512]
                    hi_i = w99[64:99, 1536 + g8 * 512:1536 + (g8 + 1) * 512]
                    r2 = lambda ap: ap.rearrange("q (j m) -> q j m", j=4)
                    nc.vector.tensor_copy(r2(lo), wpv[0:HK, :, 0, :])
                    nc.vector.tensor_copy(r2(lo_i), wpv[0:HK, :, 1, :])
                    nc.vector.tensor_copy(r2(hi), wpv[64:99, :, 0, :])
                    nc.vector.tensor_copy(r2(hi_i), wpv[64:99, :, 1, :])

                # ---- stage 2d: F groups (4 kernels) + squares ----
                for rnd in range(3):
                    fpa = fps.tile([128, 1024], f32, tag="fp", name="fpa")
                    fpb = fps.tile([128, 1024], f32, tag="fp", name="fpb")
                    ga, gb = 2 * rnd, 2 * rnd + 1
                    nc.tensor.matmul(fpa[:, 0:512], cc99r,
                                     w99[:, ga * 512:(ga + 1) * 512],
                                     start=True, stop=True)
                    nc.tensor.matmul(fpb[:, 0:512], cc99r,
                                     w99[:, gb * 512:(gb + 1) * 512],
                                     start=True, stop=True)
                    nc.tensor.matmul(fpa[:, 512:1024], cc99i,
                                     w99[:, ga * 512:(ga + 1) * 512],
                                     start=True, stop=True)
                    nc.tensor.matmul(fpb[:, 512:1024], cc99i,
                                     w99[:, gb * 512:(gb + 1) * 512],
                                     start=True, stop=True)
                    nc.scalar.activation(sq[ga][:], fpa[:], AF.Square)
                    nc.scalar.activation(sq[gb][:], fpb[:], AF.Square)

            # ---- intensity sum: bf16 tree; fold groups 0-3 early so only
            # v3's fold chain trails the last square ----
            v1 = spool.tile([128, 1024], bf16, tag="v12", name="v1")
            v2 = spool.tile([128, 1024], bf16, tag="v12", name="v2")
            v3 = spool.tile([128, 1024], bf16, tag="v34", name="v3")
            v4 = spool.tile([128, 1024], bf16, tag="v34", name="v4")
            nc.vector.tensor_add(v1[:], sq[0][:], sq[1][:])
            nc.vector.tensor_add(v2[:], sq[2][:], sq[3][:])
            nc.vector.tensor_add(v4[:], v1[:], v2[:])
            pa = wpool.tile([128, 512], f32)
            pb = wpool.tile([128, 256], f32)
            pc_ = wpool.tile([128, 128], f32)
            nc.vector.tensor_add(pa[:], v4[:, 0:512], v4[:, 512:1024])
            nc.vector.tensor_add(pb[:], pa[:, 0:256], pa[:, 256:512])
            nc.vector.tensor_add(pc_[:], pb[:, 0:128], pb[:, 128:256])
            nc.vector.tensor_add(v3[:], sq[4][:], sq[5][:])
            qa = wpool.tile([128, 512], f32)
            qb = wpool.tile([128, 256], f32)
            qc = wpool.tile([128, 128], f32)
            nc.vector.tensor_add(qa[:], v3[:, 0:512], v3[:, 512:1024])
            nc.vector.tensor_add(qb[:], qa[:, 0:256], qa[:, 256:512])
            nc.vector.tensor_add(qc[:], qb[:, 0:128], qb[:, 128:256])
            aer_cb = wpool.tile([128, 128], bf16)
            nc.vector.tensor_add(aer_cb[:], pc_[:], qc[:])

            # ---- stage 5: aerial_half = U_h @ aer_c @ U^T (bf16 matmuls) ----
            z_sb = wpool.tile([128, 512], bf16)
            with tc.tile_pool(name="zps", bufs=1, space=bass.MemorySpace.PSUM) as zps:
                zp = zps.tile([128, 512], f32)
                nc.tensor.matmul(zp[:], aer_cb[:], uht, start=True, stop=True)
                nc.scalar.copy(z_sb[:], zp[:])

            with tc.tile_pool(name="aps", bufs=2, space=bass.MemorySpace.PSUM) as aps:
                for t in range(4):
                    ap_t = aps.tile([128, N], f32)
                    nc.tensor.matmul(ap_t[:, 0:512],
                                     z_sb[:, t * 128:(t + 1) * 128],
                                     ut[:, 0:512], start=True, stop=True)
                    nc.tensor.matmul(ap_t[:, 512:1024],
                                     z_sb[:, t * 128:(t + 1) * 128],
                                     ut[:, 512:1024], start=True, stop=True)
                    aer_sb = opool.tile([128, N], bf16, tag="out", name="aer_sb")
                    if t == 0:
                        nc.scalar.copy(aer_sb[:], ap_t[:])
                    else:
                        nc.vector.tensor_copy(aer_sb[:], ap_t[:])
                    nc.sync.dma_start(aerial_d[t * 128:(t + 1) * 128, :], aer_sb[:])

    nc.compile()
    return nc


_CACHE = {}


def _get_program():
    if "nc" not in _CACHE:
        _CACHE["nc"] = _build_program()
    return _CACHE["nc"]


def _prep_inputs(mask, kernels, scales):
    import ml_dtypes
    bf = ml_dtypes.bfloat16

    atp, ctp99, cc99, ut, U = _host_matrices()

    kers = kernels.astype(np.complex128) * np.sqrt(scales.astype(np.float64))[:, None, None]
    ktR = np.ascontiguousarray(
        kers.real.astype(np.float32).transpose(2, 0, 1).reshape(HK, K * HK))
    ktI = np.ascontiguousarray(
        kers.imag.astype(np.float32).transpose(2, 0, 1).reshape(HK, K * HK))
    # 99-row / 99-col pair-block layout: block p holds kernels (2p, 2p+1) at
    # cols 0:35 / 64:99; rows 0:35 multiply M (kA top), rows 64:99 the swap.
    kri = np.zeros((99, 2 * 12 * 99), np.float32)
    for p in range(12):
        for side, k in ((0, 2 * p), (64, 2 * p + 1)):
            c = p * 99 + side
            kri[0:HK, c:c + HK] = ktR[:, k * HK:(k + 1) * HK]        # t1 top: Kr
            kri[64:99, c:c + HK] = ktI[:, k * HK:(k + 1) * HK]       # t1 bot: Ki
            kri[0:HK, 1188 + c:1188 + c + HK] = ktI[:, k * HK:(k + 1) * HK]
            kri[64:99, 1188 + c:1188 + c + HK] = ktR[:, k * HK:(k + 1) * HK]
    kri = kri.astype(bf)
    # atp packed for straight DMA: atp_p[p, c*70+u] = atp[c*128+p, u]
    atp = np.ascontiguousarray(
        atp.reshape(8, 128, 2 * HK).transpose(1, 0, 2).reshape(128, 8 * 2 * HK))
    cc = np.concatenate([ctp99, cc99], axis=1).astype(bf)      # [99, 512]
    uh = [np.ascontiguousarray(U[h * 512:(h + 1) * 512, :].T) for h in range(2)]
    uc = [np.concatenate([uh[h], ut], axis=1).astype(bf) for h in range(2)]
    atp_bf = atp.astype(bf)
    mask_bf = np.asarray(mask, np.float32).astype(bf)
    return mask_bf, atp_bf, kri, cc, uc


# ---------------------------------------------------------------- entry point
def kernel(mask, kernels, kernels_ct, scales):
    """Full inputs in, full outputs out.  Shards over 8 NeuronCores internally."""
    from concourse.bass_utils import run_bass_kernel_spmd

    kernels = np.asarray(kernels, np.complex64)
    scales = np.asarray(scales, np.float32)
    mask_bf, atp_bf, kri, cc, uc = _prep_inputs(mask, kernels, scales)

    nc = _get_program()
    in_maps = []
    for c in range(8):
        b, h = c // 2, c % 2
        in_maps.append({
            "x": mask_bf[b],
            "atp": atp_bf,
            "kri": kri,
            "cc": cc,
            "uc": uc[h],
        })

    trace = bool(int(os.environ.get("BASS_KERNEL_TRACE", "0")))
    res = run_bass_kernel_spmd(nc, in_maps, core_ids=list(range(8)), trace=trace)
    _CACHE["last_results"] = res

    aerial = np.empty((B, N, N), np.float32)
    for c in range(8):
        b, h = c // 2, c % 2
        aerial[b, h * 512:(h + 1) * 512, :] = \
            np.asarray(res.results[c]["aerial"]).astype(np.float32)
    resist = (1.0 / (1.0 + np.exp(
        -RESIST_STEEPNESS * (aerial.astype(np.float64) - RESIST_THRESHOLD)
    ))).astype(np.float32)
    printed = (aerial > RESIST_THRESHOLD).astype(np.float32)
    return aerial, resist, printed
